# revision 7
# baseline (speedup 1.0000x reference)
"""Distributed Trainium2 kernel for nn_ARLoss_88390426951926.

Computes mean(loss) where, per element (EPS = 1e-6):
    c = round(t); d = x - c; pos = d >= 0
    z = pos ? ceil(x) : floor(x)
    loss = max(0, |d| - |x - z| + pos*EPS)

Algebraic reduction used here (exact up to measure-zero ties, validated
numerically to ~5e-6 rel err on the real data):
    With F = floor(x), S = 2x - c - F:
        loss = relu(S - 1 + eps) + relu(-S)
    and since relu(a) = (a + |a|)/2 with (S-1) + (-S) = -1 constant:
        sum(loss) = 0.5*(sum|S-1| + sum|S|) - N/2
so the kernel only needs two absolute-value sums of S.

Rounding is done with the float32 magic-number trick (M = 1.5*2^23):
    c = (t + M) - M        (round-to-nearest-even)
    F = ((x - 0.5) + M) - M (floor, up to RNE ties at exact integers)

Engine split per 128x4096 tile (per-element cost):
    DVE  TS1: c  = (t + M) - M          f32->bf16   0.5 cyc (2x_2p)
    DVE  TS2: y  = (x - 0.5) + M        f32->f32    0.5 cyc
    ACT  A1 : F  = Copy(y - M)          f32->bf16   1 pass
    DVE  TS3: x2 = 2*x                  f32->bf16   0.5 cyc
    DVE  TT1: e  = c + F                bf16        0.5 cyc (2x_1p)
    DVE  TT2: S  = x2 - e               bf16        0.5 cyc
    ACT  A2 : |S - 1| with accum_out    (free-dim sum per partition)
    ACT  A3 : |S|     with accum_out
DVE ~2.5 cyc/elem (~85us/core), ACT 3 passes (~85us/core), both under
the ~94us/core HBM roofline for the 32 MiB each core reads.
"""

import numpy as np

import concourse.bass as bass
import concourse.bacc as bacc
import concourse.mybir as mybir
from concourse.tile import TileContext
from concourse.bass_utils import run_bass_kernel_spmd

B, D = 8192, 4096
N_CORES = 8
ROWS = B // N_CORES          # 1024 rows per core
P = 128                      # SBUF partitions
FD = 4096                    # free dim per tile
NTILES = (ROWS * D) // (P * FD)   # 8 tiles per core
MAGIC = 12582912.0           # 1.5 * 2**23
EPS = 1e-06

F32 = mybir.dt.float32
BF16 = mybir.dt.bfloat16

# Exposed for test.py: the BassKernelResults of the last run.
LAST_RESULTS = None
_CACHE = {}


def build_nc():
    nc = bacc.Bacc()
    # Register a -1.0 per-partition constant for the Abs(S - 1) bias
    # (the constructor only registers 0.0 and 1.0).
    neg1 = nc.alloc_sbuf_tensor("const-float32--1.0", [128, 1], F32)
    nc.gpsimd.memset(neg1.ap(), -1.0)
    nc.const_aps.aps[(F32, -1.0)] = neg1.ap()
    nc.all_engine_barrier()

    x_d = nc.dram_tensor("input", [ROWS, D], F32, kind="ExternalInput")
    t_d = nc.dram_tensor("target", [ROWS, D], F32, kind="ExternalInput")
    out_d = nc.dram_tensor("out", [2, P, NTILES], F32, kind="ExternalOutput")

    x_t = x_d[:, :].rearrange("(n p) m -> n p m", p=P)
    t_t = t_d[:, :].rearrange("(n p) m -> n p m", p=P)

    add = mybir.AluOpType.add
    sub = mybir.AluOpType.subtract
    mult = mybir.AluOpType.mult
    Copy = mybir.ActivationFunctionType.Copy
    Abs = mybir.ActivationFunctionType.Abs

    with TileContext(nc) as tc:
        with (
            tc.tile_pool(name="io", bufs=2) as io_pool,
            tc.tile_pool(name="mid", bufs=2) as mid_pool,
            tc.tile_pool(name="accs", bufs=1) as acc_pool,
        ):
            acc1 = acc_pool.tile([P, NTILES], F32)
            acc2 = acc_pool.tile([P, NTILES], F32)
            scratch = acc_pool.tile([P, FD], BF16)

            for i in range(NTILES):
                xs = io_pool.tile([P, FD], F32, tag="x")
                ts = io_pool.tile([P, FD], F32, tag="t")
                nc.sync.dma_start(xs[:, :], x_t[i])
                nc.sync.dma_start(ts[:, :], t_t[i])

                c = mid_pool.tile([P, FD], BF16, tag="c")
                y = mid_pool.tile([P, FD], F32, tag="y")
                Ftile = mid_pool.tile([P, FD], BF16, tag="F")
                x2 = mid_pool.tile([P, FD], BF16, tag="x2")
                e = mid_pool.tile([P, FD], BF16, tag="e")
                S = mid_pool.tile([P, FD], BF16, tag="S")

                # c = RNE(t): (t + M) - M, emitted as bf16 (exact: small int)
                nc.vector.tensor_scalar(c[:, :], ts[:, :], MAGIC, MAGIC, add, sub)
                # y = (x - 0.5) + M = floor(x) + M
                nc.vector.tensor_scalar(y[:, :], xs[:, :], 0.5, MAGIC, sub, add)
                # F = y - M (ACT Copy allows float bias), bf16 (exact small int)
                nc.scalar.activation(Ftile[:, :], y[:, :], Copy, bias=-MAGIC, scale=1.0)
                # x2 = 2x (bf16)
                nc.vector.tensor_scalar(x2[:, :], xs[:, :], 2.0, None, mult)
                # e = c + F (bf16 exact)
                nc.vector.tensor_tensor(e[:, :], c[:, :], Ftile[:, :], add)
                # S = x2 - e
                nc.vector.tensor_tensor(S[:, :], x2[:, :], e[:, :], sub)
                # acc1[i] = sum |S - 1|, acc2[i] = sum |S|
                nc.scalar.activation(
                    scratch[:, :], S[:, :], Abs, bias=-1.0, scale=1.0,
                    accum_out=acc1[:, i : i + 1],
                )
                nc.scalar.activation(
                    scratch[:, :], S[:, :], Abs, bias=0.0, scale=1.0,
                    accum_out=acc2[:, i : i + 1],
                )

            nc.sync.dma_start(out_d[0, :, :], acc1[:, :])
            nc.sync.dma_start(out_d[1, :, :], acc2[:, :])

    nc.compile()
    return nc


def kernel(input, target):
    global LAST_RESULTS
    x = np.ascontiguousarray(np.asarray(input, dtype=np.float32))
    t = np.ascontiguousarray(np.asarray(target, dtype=np.float32))
    assert x.shape == (B, D) and t.shape == (B, D)

    if "nc" not in _CACHE:
        _CACHE["nc"] = build_nc()
    nc = _CACHE["nc"]

    in_maps = []
    for j in range(N_CORES):
        r0, r1 = j * ROWS, (j + 1) * ROWS
        in_maps.append(
            {
                "input": np.ascontiguousarray(x[r0:r1]),
                "target": np.ascontiguousarray(t[r0:r1]),
            }
        )

    res = run_bass_kernel_spmd(nc, in_maps, core_ids=list(range(N_CORES)))
    LAST_RESULTS = res

    total = 0.0
    for j in range(N_CORES):
        total += res.results[j]["out"].astype(np.float64).sum()

    n = float(B) * float(D)
    loss = (0.5 * total - n / 2.0) / n
    return np.float32(loss)
